# revision 2
# baseline (speedup 1.0000x reference)
"""Maxwell rheological model kernel for Trainium2 (8 NeuronCores, SPMD).

Recurrence per batch row (a = E/ETA = 2, E_INFTY = 1, E = 2):
    gamma[0] = 0
    gamma[n+1] = (1 - 2*dt[n]) * gamma[n] + 2*dt[n] * eps[n]
    sigma[n+1] = 3*eps[n+1] - 2*gamma[n+1];  sigma[0] = 0

Mapped onto the DVE TensorTensorScan instruction with g = 2*gamma:
    c[n] = 1 - 2*dt[n]          (ACT engine)
    d[n] = 4*dt[n]*eps[n]       (DVE scalar_tensor_tensor)
    g[n] = c[n]*g[n-1] + d[n]   (DVE tensor_tensor_scan, init 0)
    sigma[:, 1:] = 3*eps[:, 1:] - g[:, :-1]

Batch is sharded across 8 cores (data parallel, no collectives).
"""

import sys

if "/opt/trn_rl_repo" not in sys.path:
    sys.path.insert(0, "/opt/trn_rl_repo")

import numpy as np

import concourse.bacc as bacc
import concourse.mybir as mybir
from concourse.bass_utils import run_bass_kernel_spmd
from concourse.tile import TileContext

B, T = 16384, 2048
N_CORES = 8
B_CORE = B // N_CORES
P = 128
N_STRIPS = B_CORE // P

_prog = None


def _build():
    f32 = mybir.dt.float32
    Alu = mybir.AluOpType
    nc = bacc.Bacc("TRN2", target_bir_lowering=False, debug=False, num_devices=N_CORES)
    strains = nc.dram_tensor("strains", [B_CORE, T], f32, kind="ExternalInput").ap()
    dts = nc.dram_tensor("dts", [B_CORE, T], f32, kind="ExternalInput").ap()
    out = nc.dram_tensor("out", [B_CORE, T], f32, kind="ExternalOutput").ap()
    with TileContext(nc) as tc:
        with (
            tc.tile_pool(name="pin", bufs=5) as pin,
            tc.tile_pool(name="pmid", bufs=2) as pmid,
            tc.tile_pool(name="pout", bufs=3) as pout,
        ):
            for i in range(N_STRIPS):
                r0 = i * P
                dt_t = pin.tile([P, T], f32, tag="dt")
                ep_t = pin.tile([P, T], f32, tag="eps")
                nc.sync.dma_start(out=dt_t[:], in_=dts[r0 : r0 + P])
                nc.sync.dma_start(out=ep_t[:], in_=strains[r0 : r0 + P])
                c_t = pmid.tile([P, T - 1], f32, tag="c")
                e3_t = pmid.tile([P, T - 1], f32, tag="e3")
                d_t = pmid.tile([P, T - 1], f32, tag="d")
                g_t = pmid.tile([P, T - 1], f32, tag="g")
                s_t = pout.tile([P, T], f32, tag="sig")
                # ACT: c = 1 - 2*dt (cols 0..T-2); e3 = 3*eps (cols 1..T-1)
                nc.scalar.activation(
                    out=c_t[:],
                    in_=dt_t[:, : T - 1],
                    func=mybir.ActivationFunctionType.Copy,
                    scale=-2.0,
                    bias=1.0,
                )
                nc.scalar.activation(
                    out=e3_t[:],
                    in_=ep_t[:, 1:],
                    func=mybir.ActivationFunctionType.Copy,
                    scale=3.0,
                )
                # DVE: d = (dt*4)*eps; g = scan(c, d)  [g = 2*gamma_{n+1}]
                nc.vector.scalar_tensor_tensor(
                    out=d_t[:],
                    in0=dt_t[:, : T - 1],
                    scalar=4.0,
                    in1=ep_t[:, : T - 1],
                    op0=Alu.mult,
                    op1=Alu.mult,
                )
                nc.vector.tensor_tensor_scan(
                    out=g_t[:],
                    data0=c_t[:],
                    data1=d_t[:],
                    initial=0.0,
                    op0=Alu.mult,
                    op1=Alu.add,
                )
                # Pool: sigma[:,1:] = e3 - g
                nc.gpsimd.tensor_tensor(
                    out=s_t[:, 1:], in0=e3_t[:], in1=g_t[:], op=Alu.subtract
                )
                nc.gpsimd.memset(s_t[:, 0:1], 0.0)
                nc.sync.dma_start(out=out[r0 : r0 + P], in_=s_t[:])
    nc.compile()
    return nc


def _get_prog():
    global _prog
    if _prog is None:
        _prog = _build()
    return _prog


def _run(strains, dts, **kwargs):
    nc = _get_prog()
    ss = np.split(np.ascontiguousarray(strains, dtype=np.float32), N_CORES, axis=0)
    ds = np.split(np.ascontiguousarray(dts, dtype=np.float32), N_CORES, axis=0)
    in_maps = [{"strains": s, "dts": d} for s, d in zip(ss, ds)]
    res = run_bass_kernel_spmd(nc, in_maps, core_ids=list(range(N_CORES)), **kwargs)
    full = np.concatenate([r["out"] for r in res.results], axis=0)
    return full, res


def kernel(strains, dts):
    out, _ = _run(strains, dts)
    return out


if __name__ == "__main__":
    rng = np.random.default_rng(0)
    eps = rng.standard_normal((B, T), dtype=np.float32)
    dts = rng.random((B, T), dtype=np.float32)
    out = kernel(eps, dts)
    print("ran ok", out.shape, out.dtype)


# revision 3
# speedup vs baseline: 1.0322x; 1.0322x over previous
"""Maxwell rheological model kernel for Trainium2 (8 NeuronCores, SPMD).

Recurrence per batch row (a = E/ETA = 2, E_INFTY = 1, E = 2):
    gamma[0] = 0
    gamma[n+1] = (1 - 2*dt[n]) * gamma[n] + 2*dt[n] * eps[n]
    sigma[n+1] = 3*eps[n+1] - 2*gamma[n+1];  sigma[0] = 0

Mapped onto the DVE TensorTensorScan instruction with g = 2*gamma:
    c[n] = 1 - 2*dt[n]          (ACT engine)
    d[n] = 4*dt[n]*eps[n]       (DVE scalar_tensor_tensor)
    g[n] = c[n]*g[n-1] + d[n]   (DVE tensor_tensor_scan, init 0)
    sigma[:, 1:] = 3*eps[:, 1:] - g[:, :-1]

Batch is sharded across 8 cores (data parallel, no collectives).
"""

import sys

if "/opt/trn_rl_repo" not in sys.path:
    sys.path.insert(0, "/opt/trn_rl_repo")

import numpy as np

import concourse.bacc as bacc
import concourse.mybir as mybir
from concourse.bass_utils import run_bass_kernel_spmd
from concourse.tile import TileContext

B, T = 16384, 2048
N_CORES = 8
B_CORE = B // N_CORES
P = 128
N_STRIPS = B_CORE // P

_prog = None


def _build():
    f32 = mybir.dt.float32
    Alu = mybir.AluOpType
    nc = bacc.Bacc("TRN2", target_bir_lowering=False, debug=False, num_devices=N_CORES)
    strains = nc.dram_tensor("strains", [B_CORE, T], f32, kind="ExternalInput").ap()
    dts = nc.dram_tensor("dts", [B_CORE, T], f32, kind="ExternalInput").ap()
    out = nc.dram_tensor("out", [B_CORE, T], f32, kind="ExternalOutput").ap()
    with TileContext(nc) as tc:
        with (
            tc.tile_pool(name="pin", bufs=4) as pin,
            tc.tile_pool(name="pmid", bufs=3) as pmid,
            tc.tile_pool(name="pout", bufs=5) as pout,
        ):
            for i in range(N_STRIPS):
                r0 = i * P
                dt_t = pin.tile([P, T], f32, tag="dt")
                ep_t = pin.tile([P, T], f32, tag="eps")
                nc.sync.dma_start(out=dt_t[:], in_=dts[r0 : r0 + P])
                nc.sync.dma_start(out=ep_t[:], in_=strains[r0 : r0 + P])
                c_t = pmid.tile([P, T - 1], f32, tag="c")
                d_t = pmid.tile([P, T - 1], f32, tag="d")
                g_t = pmid.tile([P, T - 1], f32, tag="g")
                s_t = pout.tile([P, T], f32, tag="sig")
                # ACT: c = 1 - 2*dt (cols 0..T-2); sig[:,1:] = 3*eps[:,1:]
                nc.scalar.activation(
                    out=c_t[:],
                    in_=dt_t[:, : T - 1],
                    func=mybir.ActivationFunctionType.Copy,
                    scale=-2.0,
                    bias=1.0,
                )
                nc.scalar.activation(
                    out=s_t[:, 1:],
                    in_=ep_t[:, 1:],
                    func=mybir.ActivationFunctionType.Copy,
                    scale=3.0,
                )
                nc.gpsimd.memset(s_t[:, 0:1], 0.0)
                # DVE: d = (dt*4)*eps; g = scan(c, d)  [g = 2*gamma_{n+1}]
                nc.vector.scalar_tensor_tensor(
                    out=d_t[:],
                    in0=dt_t[:, : T - 1],
                    scalar=4.0,
                    in1=ep_t[:, : T - 1],
                    op0=Alu.mult,
                    op1=Alu.mult,
                )
                nc.vector.tensor_tensor_scan(
                    out=g_t[:],
                    data0=c_t[:],
                    data1=d_t[:],
                    initial=0.0,
                    op0=Alu.mult,
                    op1=Alu.add,
                )
                # Pool: sig[:,1:] -= g  (in place)
                nc.gpsimd.tensor_tensor(
                    out=s_t[:, 1:], in0=s_t[:, 1:], in1=g_t[:], op=Alu.subtract
                )
                nc.sync.dma_start(out=out[r0 : r0 + P], in_=s_t[:])
    nc.compile()
    return nc


def _get_prog():
    global _prog
    if _prog is None:
        _prog = _build()
    return _prog


def _run(strains, dts, **kwargs):
    nc = _get_prog()
    ss = np.split(np.ascontiguousarray(strains, dtype=np.float32), N_CORES, axis=0)
    ds = np.split(np.ascontiguousarray(dts, dtype=np.float32), N_CORES, axis=0)
    in_maps = [{"strains": s, "dts": d} for s, d in zip(ss, ds)]
    res = run_bass_kernel_spmd(nc, in_maps, core_ids=list(range(N_CORES)), **kwargs)
    full = np.concatenate([r["out"] for r in res.results], axis=0)
    return full, res


def kernel(strains, dts):
    out, _ = _run(strains, dts)
    return out


if __name__ == "__main__":
    rng = np.random.default_rng(0)
    eps = rng.standard_normal((B, T), dtype=np.float32)
    dts = rng.random((B, T), dtype=np.float32)
    out = kernel(eps, dts)
    print("ran ok", out.shape, out.dtype)


# revision 5
# speedup vs baseline: 1.0547x; 1.0218x over previous
"""Maxwell rheological model kernel for Trainium2 (8 NeuronCores, SPMD).

Recurrence per batch row (a = E/ETA = 2, E_INFTY = 1, E = 2):
    gamma[0] = 0
    gamma[n+1] = (1 - 2*dt[n]) * gamma[n] + 2*dt[n] * eps[n]
    sigma[n+1] = 3*eps[n+1] - 2*gamma[n+1];  sigma[0] = 0

Mapped onto the DVE TensorTensorScan instruction with g = 2*gamma:
    c[n] = 1 - 2*dt[n]          (ACT engine)
    d[n] = 4*dt[n]*eps[n]       (DVE scalar_tensor_tensor)
    g[n] = c[n]*g[n-1] + d[n]   (DVE tensor_tensor_scan, init 0)
    sigma[:, 1:] = 3*eps[:, 1:] - g[:, :-1]

Batch is sharded across 8 cores (data parallel, no collectives).
"""

import sys

if "/opt/trn_rl_repo" not in sys.path:
    sys.path.insert(0, "/opt/trn_rl_repo")

import numpy as np

import concourse.bacc as bacc
import concourse.mybir as mybir
from concourse.bass_utils import run_bass_kernel_spmd
from concourse.tile import TileContext

B, T = 16384, 2048
N_CORES = 8
B_CORE = B // N_CORES
P = 128
N_STRIPS = B_CORE // P

_prog = None


def _build():
    f32 = mybir.dt.float32
    Alu = mybir.AluOpType
    nc = bacc.Bacc("TRN2", target_bir_lowering=False, debug=False, num_devices=N_CORES)
    strains = nc.dram_tensor("strains", [B_CORE, T], f32, kind="ExternalInput").ap()
    dts = nc.dram_tensor("dts", [B_CORE, T], f32, kind="ExternalInput").ap()
    out = nc.dram_tensor("out", [B_CORE, T], f32, kind="ExternalOutput").ap()
    with TileContext(nc) as tc:
        with (
            tc.tile_pool(name="pin", bufs=4) as pin,
            tc.tile_pool(name="pmid", bufs=4) as pmid,
            tc.tile_pool(name="pout", bufs=4) as pout,
        ):
            for i in range(N_STRIPS):
                r0 = i * P
                dt_t = pin.tile([P, T], f32, tag="dt")
                ep_t = pin.tile([P, T], f32, tag="eps")
                nc.sync.dma_start(out=dt_t[:], in_=dts[r0 : r0 + P])
                nc.sync.dma_start(out=ep_t[:], in_=strains[r0 : r0 + P])
                c_t = pmid.tile([P, T - 1], f32, tag="c")
                d_t = pmid.tile([P, T - 1], f32, tag="d")
                g_t = pmid.tile([P, T - 1], f32, tag="g")
                s_t = pout.tile([P, T], f32, tag="sig")
                # ACT: c = 1 - 2*dt (cols 0..T-2); sig[:,1:] = 3*eps[:,1:];
                # sig[:,0] = 0 (scale=0 copy) — all sig writes on one engine.
                nc.scalar.activation(
                    out=c_t[:],
                    in_=dt_t[:, : T - 1],
                    func=mybir.ActivationFunctionType.Copy,
                    scale=-2.0,
                    bias=1.0,
                )
                nc.scalar.activation(
                    out=s_t[:, 1:],
                    in_=ep_t[:, 1:],
                    func=mybir.ActivationFunctionType.Copy,
                    scale=3.0,
                )
                nc.scalar.activation(
                    out=s_t[:, 0:1],
                    in_=ep_t[:, 0:1],
                    func=mybir.ActivationFunctionType.Copy,
                    scale=0.0,
                )
                # DVE: d = (dt*4)*eps; g = scan(c, d)  [g = 2*gamma_{n+1}]
                nc.vector.scalar_tensor_tensor(
                    out=d_t[:],
                    in0=dt_t[:, : T - 1],
                    scalar=4.0,
                    in1=ep_t[:, : T - 1],
                    op0=Alu.mult,
                    op1=Alu.mult,
                )
                nc.vector.tensor_tensor_scan(
                    out=g_t[:],
                    data0=c_t[:],
                    data1=d_t[:],
                    initial=0.0,
                    op0=Alu.mult,
                    op1=Alu.add,
                )
                # Pool: sig[:,1:] -= g  (in place)
                nc.gpsimd.tensor_tensor(
                    out=s_t[:, 1:], in0=s_t[:, 1:], in1=g_t[:], op=Alu.subtract
                )
                # Store issued from gpsimd (SWDGE): Pool just finished sigma,
                # so the store's wait is already satisfied and store issues
                # don't head-of-line block load issues on Sync.
                nc.gpsimd.dma_start(out=out[r0 : r0 + P], in_=s_t[:])
    nc.compile()
    return nc


def _get_prog():
    global _prog
    if _prog is None:
        _prog = _build()
    return _prog


def _run(strains, dts, **kwargs):
    nc = _get_prog()
    ss = np.split(np.ascontiguousarray(strains, dtype=np.float32), N_CORES, axis=0)
    ds = np.split(np.ascontiguousarray(dts, dtype=np.float32), N_CORES, axis=0)
    in_maps = [{"strains": s, "dts": d} for s, d in zip(ss, ds)]
    res = run_bass_kernel_spmd(nc, in_maps, core_ids=list(range(N_CORES)), **kwargs)
    full = np.concatenate([r["out"] for r in res.results], axis=0)
    return full, res


def kernel(strains, dts):
    out, _ = _run(strains, dts)
    return out


if __name__ == "__main__":
    rng = np.random.default_rng(0)
    eps = rng.standard_normal((B, T), dtype=np.float32)
    dts = rng.random((B, T), dtype=np.float32)
    out = kernel(eps, dts)
    print("ran ok", out.shape, out.dtype)


# revision 6
# speedup vs baseline: 1.2883x; 1.2215x over previous
"""Maxwell rheological model kernel for Trainium2 (8 NeuronCores, SPMD).

Recurrence per batch row (a = E/ETA = 2, E_INFTY = 1, E = 2):
    gamma[0] = 0
    gamma[n+1] = (1 - 2*dt[n]) * gamma[n] + 2*dt[n] * eps[n]
    sigma[n+1] = 3*eps[n+1] - 2*gamma[n+1];  sigma[0] = 0

Mapped onto the DVE TensorTensorScan instruction with g = 2*gamma:
    c[n] = 1 - 2*dt[n]          (ACT engine)
    d[n] = 4*dt[n]*eps[n]       (DVE scalar_tensor_tensor)
    g[n] = c[n]*g[n-1] + d[n]   (DVE tensor_tensor_scan, init 0)
    sigma[:, 1:] = 3*eps[:, 1:] - g[:, :-1]

Batch is sharded across 8 cores (data parallel, no collectives).
"""

import sys

if "/opt/trn_rl_repo" not in sys.path:
    sys.path.insert(0, "/opt/trn_rl_repo")

import numpy as np

import concourse.bacc as bacc
import concourse.mybir as mybir
from concourse.bass_utils import run_bass_kernel_spmd
from concourse.tile import TileContext

B, T = 16384, 2048
N_CORES = 8
B_CORE = B // N_CORES
P = 128
N_STRIPS = B_CORE // P

_prog = None


def _build():
    f32 = mybir.dt.float32
    Alu = mybir.AluOpType
    nc = bacc.Bacc("TRN2", target_bir_lowering=False, debug=False, num_devices=N_CORES)
    strains = nc.dram_tensor("strains", [B_CORE, T], f32, kind="ExternalInput").ap()
    dts = nc.dram_tensor("dts", [B_CORE, T], f32, kind="ExternalInput").ap()
    out = nc.dram_tensor("out", [B_CORE, T], f32, kind="ExternalOutput").ap()
    with TileContext(nc) as tc:
        with (
            tc.tile_pool(name="pin", bufs=4) as pin,
            tc.tile_pool(name="pc", bufs=4) as pc,
            tc.tile_pool(name="pmid", bufs=3) as pmid,
            tc.tile_pool(name="pout", bufs=4) as pout,
        ):
            for i in range(N_STRIPS):
                r0 = i * P
                dt_t = pin.tile([P, T], f32, tag="dt")
                ep_t = pin.tile([P, T], f32, tag="eps")
                nc.sync.dma_start(out=dt_t[:], in_=dts[r0 : r0 + P])
                nc.sync.dma_start(out=ep_t[:], in_=strains[r0 : r0 + P])
                c_t = pc.tile([P, T - 1], f32, tag="c")
                d_t = pmid.tile([P, T - 1], f32, tag="d")
                g_t = pmid.tile([P, T - 1], f32, tag="g")
                s_t = pout.tile([P, T], f32, tag="sig")
                # ACT: c = 1 - 2*dt (cols 0..T-2); sig[:,0] = 0
                nc.scalar.activation(
                    out=c_t[:],
                    in_=dt_t[:, : T - 1],
                    func=mybir.ActivationFunctionType.Copy,
                    scale=-2.0,
                    bias=1.0,
                )
                nc.scalar.activation(
                    out=s_t[:, 0:1],
                    in_=dt_t[:, 0:1],
                    func=mybir.ActivationFunctionType.Copy,
                    scale=0.0,
                )
                # DVE (all three: keeps the shared DVE/Pool datapath to
                # exactly 4 pass-equivalents per strip, no port contention):
                # d = (dt*4)*eps; g = scan(c, d) [= 2*gamma_{n+1}];
                # sig[:,1:] = 3*eps[:,1:] - g
                nc.vector.scalar_tensor_tensor(
                    out=d_t[:],
                    in0=dt_t[:, : T - 1],
                    scalar=4.0,
                    in1=ep_t[:, : T - 1],
                    op0=Alu.mult,
                    op1=Alu.mult,
                )
                nc.vector.tensor_tensor_scan(
                    out=g_t[:],
                    data0=c_t[:],
                    data1=d_t[:],
                    initial=0.0,
                    op0=Alu.mult,
                    op1=Alu.add,
                )
                nc.vector.scalar_tensor_tensor(
                    out=s_t[:, 1:],
                    in0=ep_t[:, 1:],
                    scalar=3.0,
                    in1=g_t[:],
                    op0=Alu.mult,
                    op1=Alu.subtract,
                )
                # Store issued from ACT's HWDGE ring: keeps Sync purely for
                # loads so store waits don't block load issues.
                nc.scalar.dma_start(out=out[r0 : r0 + P], in_=s_t[:])
    nc.compile()
    return nc


def _get_prog():
    global _prog
    if _prog is None:
        _prog = _build()
    return _prog


def _run(strains, dts, **kwargs):
    nc = _get_prog()
    ss = np.split(np.ascontiguousarray(strains, dtype=np.float32), N_CORES, axis=0)
    ds = np.split(np.ascontiguousarray(dts, dtype=np.float32), N_CORES, axis=0)
    in_maps = [{"strains": s, "dts": d} for s, d in zip(ss, ds)]
    res = run_bass_kernel_spmd(nc, in_maps, core_ids=list(range(N_CORES)), **kwargs)
    full = np.concatenate([r["out"] for r in res.results], axis=0)
    return full, res


def kernel(strains, dts):
    out, _ = _run(strains, dts)
    return out


if __name__ == "__main__":
    rng = np.random.default_rng(0)
    eps = rng.standard_normal((B, T), dtype=np.float32)
    dts = rng.random((B, T), dtype=np.float32)
    out = kernel(eps, dts)
    print("ran ok", out.shape, out.dtype)


# revision 8
# speedup vs baseline: 1.2989x; 1.0082x over previous
"""Maxwell rheological model kernel for Trainium2 (8 NeuronCores, SPMD).

Recurrence per batch row (a = E/ETA = 2, E_INFTY = 1, E = 2):
    gamma[0] = 0
    gamma[n+1] = (1 - 2*dt[n]) * gamma[n] + 2*dt[n] * eps[n]
    sigma[n+1] = 3*eps[n+1] - 2*gamma[n+1];  sigma[0] = 0

Mapped onto the DVE TensorTensorScan instruction with g = 2*gamma:
    c[n] = 1 - 2*dt[n]          (ACT engine)
    d[n] = 4*dt[n]*eps[n]       (DVE scalar_tensor_tensor)
    g[n] = c[n]*g[n-1] + d[n]   (DVE tensor_tensor_scan, init 0)
    sigma[:, 1:] = 3*eps[:, 1:] - g[:, :-1]

Batch is sharded across 8 cores (data parallel, no collectives).
"""

import sys

if "/opt/trn_rl_repo" not in sys.path:
    sys.path.insert(0, "/opt/trn_rl_repo")

import numpy as np

import concourse.bacc as bacc
import concourse.mybir as mybir
from concourse.bass_utils import run_bass_kernel_spmd
from concourse.tile import TileContext

B, T = 16384, 2048
N_CORES = 8
B_CORE = B // N_CORES
P = 128
N_STRIPS = B_CORE // P

_prog = None


def _build():
    f32 = mybir.dt.float32
    Alu = mybir.AluOpType
    nc = bacc.Bacc(
        "TRN2",
        target_bir_lowering=False,
        debug=False,
        enable_asserts=False,
        num_devices=N_CORES,
    )
    strains = nc.dram_tensor("strains", [B_CORE, T], f32, kind="ExternalInput").ap()
    dts = nc.dram_tensor("dts", [B_CORE, T], f32, kind="ExternalInput").ap()
    out = nc.dram_tensor("out", [B_CORE, T], f32, kind="ExternalOutput").ap()
    with TileContext(nc) as tc:
        with (
            tc.tile_pool(name="pin", bufs=4) as pin,
            tc.tile_pool(name="pc", bufs=4) as pc,
            tc.tile_pool(name="pmid", bufs=3) as pmid,
            tc.tile_pool(name="pout", bufs=4) as pout,
        ):
            H = T // 2
            for i in range(N_STRIPS):
                r0 = i * P
                # First/last strips are processed in two column halves
                # (chained scans) to shorten the pipeline head and tail.
                split = i == 0 or i == N_STRIPS - 1
                dt_t = pin.tile([P, T], f32, tag="dt")
                ep_t = pin.tile([P, T], f32, tag="eps")
                if split:
                    nc.sync.dma_start(out=dt_t[:, :H], in_=dts[r0 : r0 + P, :H])
                    nc.sync.dma_start(out=ep_t[:, :H], in_=strains[r0 : r0 + P, :H])
                    nc.sync.dma_start(out=dt_t[:, H:], in_=dts[r0 : r0 + P, H:])
                    nc.sync.dma_start(out=ep_t[:, H:], in_=strains[r0 : r0 + P, H:])
                else:
                    nc.sync.dma_start(out=dt_t[:], in_=dts[r0 : r0 + P])
                    nc.sync.dma_start(out=ep_t[:], in_=strains[r0 : r0 + P])
                c_t = pc.tile([P, T - 1], f32, tag="c")
                d_t = pmid.tile([P, T - 1], f32, tag="d")
                g_t = pmid.tile([P, T - 1], f32, tag="g")
                s_t = pout.tile([P, T], f32, tag="sig")

                def act_c(lo, hi):
                    nc.scalar.activation(
                        out=c_t[:, lo:hi],
                        in_=dt_t[:, lo:hi],
                        func=mybir.ActivationFunctionType.Copy,
                        scale=-2.0,
                        bias=1.0,
                    )

                def dve_d(lo, hi):
                    nc.vector.scalar_tensor_tensor(
                        out=d_t[:, lo:hi],
                        in0=dt_t[:, lo:hi],
                        scalar=4.0,
                        in1=ep_t[:, lo:hi],
                        op0=Alu.mult,
                        op1=Alu.mult,
                    )

                def dve_scan(lo, hi, init):
                    nc.vector.tensor_tensor_scan(
                        out=g_t[:, lo:hi],
                        data0=c_t[:, lo:hi],
                        data1=d_t[:, lo:hi],
                        initial=init,
                        op0=Alu.mult,
                        op1=Alu.add,
                    )

                def dve_sig(lo, hi):
                    # sig[:, lo:hi] = 3*eps[:, lo:hi] - g[:, lo-1:hi-1]
                    nc.vector.scalar_tensor_tensor(
                        out=s_t[:, lo:hi],
                        in0=ep_t[:, lo:hi],
                        scalar=3.0,
                        in1=g_t[:, lo - 1 : hi - 1],
                        op0=Alu.mult,
                        op1=Alu.subtract,
                    )

                # sig[:,0] = 0
                nc.scalar.activation(
                    out=s_t[:, 0:1],
                    in_=dt_t[:, 0:1],
                    func=mybir.ActivationFunctionType.Copy,
                    scale=0.0,
                )
                if split:
                    act_c(0, H)
                    dve_d(0, H)
                    dve_scan(0, H, 0.0)
                    dve_sig(1, H)
                    nc.scalar.dma_start(
                        out=out[r0 : r0 + P, :H], in_=s_t[:, :H]
                    )
                    act_c(H, T - 1)
                    dve_d(H, T - 1)
                    dve_scan(H, T - 1, g_t[:, H - 1 : H])
                    dve_sig(H, T)
                    nc.scalar.dma_start(
                        out=out[r0 : r0 + P, H:], in_=s_t[:, H:]
                    )
                else:
                    act_c(0, T - 1)
                    dve_d(0, T - 1)
                    dve_scan(0, T - 1, 0.0)
                    dve_sig(1, T)
                    # Store issued from ACT's HWDGE ring: keeps Sync purely
                    # for loads so store waits don't block load issues.
                    nc.scalar.dma_start(out=out[r0 : r0 + P], in_=s_t[:])
    nc.compile()
    return nc


def _get_prog():
    global _prog
    if _prog is None:
        _prog = _build()
    return _prog


def _run(strains, dts, **kwargs):
    nc = _get_prog()
    ss = np.split(np.ascontiguousarray(strains, dtype=np.float32), N_CORES, axis=0)
    ds = np.split(np.ascontiguousarray(dts, dtype=np.float32), N_CORES, axis=0)
    in_maps = [{"strains": s, "dts": d} for s, d in zip(ss, ds)]
    res = run_bass_kernel_spmd(nc, in_maps, core_ids=list(range(N_CORES)), **kwargs)
    full = np.concatenate([r["out"] for r in res.results], axis=0)
    return full, res


def kernel(strains, dts):
    out, _ = _run(strains, dts)
    return out


if __name__ == "__main__":
    rng = np.random.default_rng(0)
    eps = rng.standard_normal((B, T), dtype=np.float32)
    dts = rng.random((B, T), dtype=np.float32)
    out = kernel(eps, dts)
    print("ran ok", out.shape, out.dtype)
